# revision 8
# baseline (speedup 1.0000x reference)
# Trainium2 Bass kernel for nn_Clip_Codebook (CLIP-style text transformer with
# per-layer VQ-codebook cross-attention head).
#
# Sharding: data-parallel over batch B=256 across 8 NeuronCores (32 batches
# per core); codebook and all layer weights replicated.
#
# On-device layout strategy (per core):
#   - residual stream x: fp32, token-tiled SBUF [128, 20, 512]
#     (token t = j*128 + p lives at partition p, tile j; tokens 2464..2559 pad)
#   - matmul operands are produced feature-major ("xT": [D-part, token-free])
#     via PE-transpose; LayerNorm is applied token-major with fused
#     (x - mean) * rstd on the vector engine (ln scale folded into weights).
#   - attention runs per (batch, head) in a flash-style [key, query] layout:
#     softmax denominator comes from an extra N=1 matmul against ones, no
#     max-subtraction (logits are O(5) for this model family).
#   - codebook inner product is evicted from PSUM directly through a
#     segmented (per-batch) reduce_max, so the [B,T,K] tensor is never
#     materialized.
import math
import os

import numpy as np
import ml_dtypes

import concourse.bass as bass
import concourse.bacc as bacc_mod
import concourse.mybir as mybir
import concourse.tile as tile
from concourse.bass import ds
from concourse.bass_utils import run_bass_kernel_spmd

F32 = mybir.dt.float32
BF16 = mybir.dt.bfloat16
AF = mybir.ActivationFunctionType
ALU = mybir.AluOpType
AX = mybir.AxisListType
bf16 = ml_dtypes.bfloat16

# model dims
L, T, D, H, C, K, V, E, B = 12, 77, 512, 8, 256, 2048, 49408, 512, 256
DH = D // H  # 64
NEG = -1e9
EPS = 1e-5
NCORES = 8
BC = B // NCORES      # 32 batches per core
NTOK = BC * T         # 2464 real tokens per core
NTT = (NTOK + 127) // 128  # 20 token tiles
NPAD = NTT * 128      # 2560
ND = D // 128         # 4
NKO = (4 * D) // 128  # 16 fc output tiles
NCB = K // 128        # 16 codebook tiles
NC_TILES = C // 128   # 2

# batch chunks for feature-major matmuls whose PSUM is consumed with
# per-batch segmentation (6*77=462 <= 512 fp32 PSUM bank)
CHUNKS = [(0, 6), (6, 6), (12, 6), (18, 6), (24, 6), (30, 2)]

LAST_RESULTS = None  # stash of BassKernelResults for test harnesses


def _emit(nc, n_layers):
    """Emit the full per-core program into `nc`."""
    # ---------------- DRAM I/O ----------------
    x0_d = nc.dram_tensor("x0", [128, NTT, D], F32, kind="ExternalInput")
    keep16_d = nc.dram_tensor("keep16", [128, NTT], F32, kind="ExternalInput")
    wqk_d = nc.dram_tensor("wqk", [n_layers * D, 2 * D], BF16, kind="ExternalInput")
    wv_d = nc.dram_tensor("wv", [n_layers * D, D], BF16, kind="ExternalInput")
    wout_d = nc.dram_tensor("wout", [n_layers * D, D], BF16, kind="ExternalInput")
    wfc_d = nc.dram_tensor("wfc", [n_layers * D, 4 * D], BF16, kind="ExternalInput")
    wproj_d = nc.dram_tensor("wproj", [n_layers * 4 * D, D], BF16, kind="ExternalInput")
    wf1_d = nc.dram_tensor("wf1", [n_layers * D, C], BF16, kind="ExternalInput")
    wf2_d = nc.dram_tensor("wf2", [n_layers * C, C], BF16, kind="ExternalInput")
    cbt_d = nc.dram_tensor("cbt", [C, K], BF16, kind="ExternalInput")
    cbk_d = nc.dram_tensor("cbk", [K, C], BF16, kind="ExternalInput")
    tproj_d = nc.dram_tensor("tproj", [D, E], BF16, kind="ExternalInput")
    ident_d = nc.dram_tensor("ident", [128, 128], BF16, kind="ExternalInput")
    mask_d = nc.dram_tensor("mask_qk", [T, T], BF16, kind="ExternalInput")
    ones_d = nc.dram_tensor("ones_col", [128, 1], BF16, kind="ExternalInput")

    local_d = nc.dram_tensor("local_out", [n_layers, BC, C], F32, kind="ExternalOutput")
    pooled_d = nc.dram_tensor("pooled_out", [128, NTT, E], F32, kind="ExternalOutput")

    with tile.TileContext(nc) as tc:
        with tc.tile_pool(name="persist", bufs=1) as persist:
            x = persist.tile([128, NTT, D], F32, name="x")
            keep16 = persist.tile([128, NTT], F32, name="keep16sb")
            ident = persist.tile([128, 128], BF16, name="identsb")
            maskqk = persist.tile([T, T], BF16, name="masksb")
            ones_c = persist.tile([128, 1], BF16, name="onessb")
            cbt_sb = persist.tile([128, NC_TILES, K], BF16, name="cbtsb")
            cbk_sb = persist.tile([128, NCB, C], BF16, name="cbksb")
            eps_sb = persist.tile([128, 1], F32, name="epssb")
            nc.vector.memset(eps_sb, EPS)
            nc._eps_sb = eps_sb

            for j in range(NTT):
                nc.sync.dma_start(out=x[:, j, :], in_=x0_d[:, j, :])
            nc.sync.dma_start(out=keep16, in_=keep16_d[:, :])
            nc.sync.dma_start(out=ident, in_=ident_d[:, :])
            nc.sync.dma_start(out=maskqk, in_=mask_d[:, :])
            nc.sync.dma_start(out=ones_c, in_=ones_d[:, :])
            for ci in range(NC_TILES):
                nc.sync.dma_start(out=cbt_sb[:, ci, :], in_=cbt_d[ds(ci * 128, 128), :])
            for ko in range(NCB):
                nc.sync.dma_start(out=cbk_sb[:, ko, :], in_=cbk_d[ds(ko * 128, 128), :])

            # zero the pad-token region so junk can't turn into NaNs
            nc.vector.memset(x[96:128, NTT - 1, :], 0.0)

            for l in range(n_layers):
                _layer(nc, tc, l, x, keep16, ident, maskqk, ones_c, cbt_sb,
                       cbk_sb, wqk_d, wv_d, wout_d, wfc_d, wproj_d, wf1_d,
                       wf2_d, local_d)

            # ---------- final LN + text projection (all tokens) ----------
            with tc.tile_pool(name="fin", bufs=2) as fin, \
                 tc.tile_pool(name="finps", bufs=2, space="PSUM") as finps:
                tp_sb = fin.tile([128, ND, E], BF16, name="tpsb")
                for di in range(ND):
                    nc.sync.dma_start(out=tp_sb[:, di, :], in_=tproj_d[ds(di * 128, 128), :])
                mv, r_ = _ln_stats(nc, fin, finps, x, "f")
                xt = fin.tile([128, ND, NPAD], BF16, name="xft")
                _center_transpose(nc, tc, fin, finps, x, mv, r_, xt, ident, "f")
                for j in range(NTT):
                    p_ps = finps.tile([128, E], F32, name="plps", tag="plps")
                    for di in range(ND):
                        nc.tensor.matmul(p_ps, lhsT=xt[:, di, ds(j * 128, 128)],
                                         rhs=tp_sb[:, di, :],
                                         start=(di == 0), stop=(di == ND - 1))
                    po = fin.tile([128, E], F32, name="poolo", tag="poolo")
                    nc.vector.tensor_copy(out=po, in_=p_ps)
                    nc.sync.dma_start(out=pooled_d[:, j, :], in_=po)
    return nc


def _ln_stats(nc, pool, psp, src, nm, nfree=D):
    """bn_stats/bn_aggr per token tile -> (mv [128,NTT,2], r [128,NTT,1])."""
    stats = pool.tile([128, NTT, 6], F32, name=f"st{nm}", tag=f"st{nm}")
    mv = pool.tile([128, NTT, 2], F32, name=f"mv{nm}", tag=f"mv{nm}")
    r_ = pool.tile([128, NTT, 1], F32, name=f"r{nm}", tag=f"r{nm}")
    for j in range(NTT):
        if src.shape == (128, NTT, nfree):
            s_ = src[:, j, :]
        else:
            s_ = src[:, j * nfree:(j + 1) * nfree]
        nc.vector.bn_stats(out=stats[:, j, :], in_=s_)
        nc.vector.bn_aggr(out=mv[:, j, :], in_=stats[:, j, :])
    # r = 1/sqrt(var + eps)
    nc.scalar.activation(out=r_, in_=mv[:, :, 1:2], func=AF.Sqrt,
                         bias=nc._eps_sb)
    nc.vector.reciprocal(out=r_, in_=r_)
    return mv, r_


def _center_transpose(nc, tc, pool, psp, src, mv, r_, dstT, ident, nm,
                      nfree=D, scale2=None):
    """dstT[di][:, j*128:...] = transpose of ((src - mean) * r) per tile (bf16).

    scale2: optional [128, NTT] per-token extra factor folded into r.
    """
    nd = nfree // 128
    for j in range(NTT):
        xh = pool.tile([128, nfree], BF16, name=f"xh{nm}", tag=f"xh{nm}")
        if scale2 is not None:
            rr = pool.tile([128, 1], F32, name=f"rr{nm}", tag=f"rr{nm}")
            nc.vector.tensor_tensor(out=rr, in0=r_[:, j, :],
                                    in1=scale2[:, j:j + 1], op=ALU.mult)
            sc2 = rr
        else:
            sc2 = r_[:, j, :]
        src_j = src[:, j, :] if src.shape == (128, NTT, nfree) \
            else src[:, j * nfree:(j + 1) * nfree]
        nc.vector.tensor_scalar(out=xh, in0=src_j, scalar1=mv[:, j, 0:1],
                                scalar2=sc2, op0=ALU.subtract, op1=ALU.mult)
        for di in range(nd):
            t_ps = psp.tile([128, 128], BF16, name=f"tp{nm}", tag=f"tp{nm}")
            nc.tensor.transpose(t_ps, xh[:, di * 128:(di + 1) * 128], ident)
            nc.vector.tensor_copy(out=dstT[:, di, ds(j * 128, 128)], in_=t_ps)


def _layer(nc, tc, l, x, keep16, ident, maskqk, ones_c, cbt_sb, cbk_sb,
           wqk_d, wv_d, wout_d, wfc_d, wproj_d, wf1_d, wf2_d, local_d):
    lD = l * D

    # ---------------- attention ----------------
    with tc.tile_pool(name="attn", bufs=1) as ap, \
         tc.tile_pool(name="attn2", bufs=2) as ap2, \
         tc.tile_pool(name="attn3", bufs=3) as ap3:
        w_qk = ap.tile([128, ND, 2 * D], BF16, name="wqksb")
        w_v = ap.tile([128, ND, D], BF16, name="wvsb")
        w_out = ap.tile([128, ND, D], BF16, name="woutsb")
        for di in range(ND):
            nc.sync.dma_start(out=w_qk[:, di, :], in_=wqk_d[ds(lD + di * 128, 128), :])
            nc.sync.dma_start(out=w_v[:, di, :], in_=wv_d[ds(lD + di * 128, 128), :])
            nc.sync.dma_start(out=w_out[:, di, :], in_=wout_d[ds(lD + di * 128, 128), :])

        with tc.tile_pool(name="lnps1", bufs=2, space="PSUM") as pln:
            mv1, r1 = _ln_stats(nc, ap2, pln, x, "1")
            x1t = ap.tile([128, ND, NPAD], BF16, name="x1t")
            _center_transpose(nc, tc, ap2, pln, x, mv1, r1, x1t, ident, "1")

        oT = ap.tile([128, ND, NPAD], BF16, name="oT")
        nc.vector.memset(oT[:, :, NTOK:], 0.0)

        aps_cm = tc.tile_pool(name="atps", bufs=2, space="PSUM")
        aps1_cm = tc.tile_pool(name="atps1", bufs=1, space="PSUM")
        aps = aps_cm.__enter__()
        aps1 = aps1_cm.__enter__()
        for (b0, nb) in CHUNKS:
            ncols = nb * T
            c0 = b0 * T
            qkt = ap2.tile([128, 8, 6 * T], BF16, name="qkt", tag="qkt")
            for do in range(8):
                qk_ps = aps.tile([128, 6 * T], F32, name="qkps", tag="qkps")
                for di in range(ND):
                    nc.tensor.matmul(qk_ps[:, :ncols],
                                     lhsT=w_qk[:, di, ds(do * 128, 128)],
                                     rhs=x1t[:, di, ds(c0, ncols)],
                                     start=(di == 0), stop=(di == ND - 1))
                nc.vector.tensor_copy(out=qkt[:, do, :ncols], in_=qk_ps[:, :ncols])
            for bi in range(nb):
                b = b0 + bi
                cb0 = bi * T
                v_ps = aps1.tile([T, D], F32, name="vps", tag="vps")
                for di in range(ND):
                    nc.tensor.matmul(v_ps, lhsT=x1t[:, di, ds(b * T, T)],
                                     rhs=w_v[:, di, :],
                                     start=(di == 0), stop=(di == ND - 1))
                v_b = ap3.tile([T, H, 65], BF16, name="vb", tag="vb")
                nc.vector.tensor_copy(
                    out=v_b[:, :, 0:64],
                    in_=v_ps.rearrange("p (h d) -> p h d", d=64))
                nc.vector.memset(v_b[:, :, 64:65], 1.0)
                o_b = ap2.tile([T, D], BF16, name="ob", tag="ob")
                for h in range(H):
                    dt_ = h // 2
                    po = (h % 2) * 64
                    s_ps = aps.tile([T, T], F32, name="sps", tag="sps")
                    nc.tensor.matmul(s_ps,
                                     lhsT=qkt[po:po + 64, 4 + dt_, ds(cb0, T)],
                                     rhs=qkt[po:po + 64, dt_, ds(cb0, T)],
                                     start=True, stop=True)
                    e_t = ap3.tile([T, T], BF16, name="et", tag="et")
                    nc.scalar.activation(out=e_t, in_=s_ps, func=AF.Exp,
                                         scale=1.0 / math.sqrt(DH))
                    a_un = ap3.tile([T, T], BF16, name="aun", tag="aun")
                    nc.vector.tensor_tensor(out=a_un, in0=e_t, in1=maskqk,
                                            op=ALU.mult)
                    o_ps = aps.tile([T, 65], F32, name="ops", tag="ops")
                    nc.tensor.matmul(o_ps, lhsT=a_un, rhs=v_b[:, h, :],
                                     start=True, stop=True)
                    rinv = ap3.tile([T, 1], F32, name="rinv", tag="rinv")
                    nc.vector.reciprocal(out=rinv, in_=o_ps[:, 64:65])
                    nc.vector.tensor_scalar(out=o_b[:, h * 64:h * 64 + 64],
                                            in0=o_ps[:, 0:64], scalar1=rinv,
                                            scalar2=None, op0=ALU.mult)
                for di in range(ND):
                    t_ps = aps1.tile([128, T], BF16, name="otps", tag="otps")
                    nc.tensor.transpose(t_ps, o_b[:, di * 128:(di + 1) * 128],
                                        ident[:T, :T])
                    nc.vector.tensor_copy(out=oT[:, di, ds(b * T, T)], in_=t_ps)

        aps1_cm.__exit__(None, None, None)
        aps_cm.__exit__(None, None, None)

        # out-projection + residual
        with tc.tile_pool(name="zpsp", bufs=2, space="PSUM") as zpool:
          for j in range(NTT):
            z_ps = zpool.tile([128, D], F32, name="zps", tag="zps")
            for di in range(ND):
                nc.tensor.matmul(z_ps, lhsT=oT[:, di, ds(j * 128, 128)],
                                 rhs=w_out[:, di, :],
                                 start=(di == 0), stop=(di == ND - 1))
            nc.vector.tensor_tensor(out=x[:, j, :], in0=z_ps, in1=x[:, j, :],
                                    op=ALU.add)

    # ---------------- MLP ----------------
    with tc.tile_pool(name="mlp", bufs=1) as mp, \
         tc.tile_pool(name="mlp2", bufs=2) as mp2:
        w_fc = mp.tile([128, ND, 4 * D], BF16, name="wfcsb")
        w_proj = mp.tile([128, NKO, D], BF16, name="wprojsb")
        for di in range(ND):
            nc.sync.dma_start(out=w_fc[:, di, :], in_=wfc_d[ds(lD + di * 128, 128), :])
        for ko in range(NKO):
            nc.sync.dma_start(out=w_proj[:, ko, :],
                              in_=wproj_d[ds(l * 4 * D + ko * 128, 128), :])

        with tc.tile_pool(name="lnps2", bufs=2, space="PSUM") as pln2:
            mv2, r2 = _ln_stats(nc, mp2, pln2, x, "2")
            x2t = mp.tile([128, ND, NPAD], BF16, name="x2t")
            _center_transpose(nc, tc, mp2, pln2, x, mv2, r2, x2t, ident, "2")

        mps_cm = tc.tile_pool(name="mlpps", bufs=2, space="PSUM")
        mps = mps_cm.__enter__()
        for ch in range(NPAD // 512):
            mt = mp2.tile([128, NKO, 512], BF16, name="mt", tag="mt")
            for ko in range(NKO):
                m_ps = mps.tile([128, 512], F32, name="mps", tag="mps")
                for di in range(ND):
                    nc.tensor.matmul(m_ps, lhsT=w_fc[:, di, ds(ko * 128, 128)],
                                     rhs=x2t[:, di, ds(ch * 512, 512)],
                                     start=(di == 0), stop=(di == ND - 1))
                nc.scalar.activation(out=mt[:, ko, :], in_=m_ps, func=AF.Silu,
                                     scale=1.702)
            for jj in range(4):
                j = ch * 4 + jj
                p_ps = mps.tile([128, D], F32, name="pps", tag="pps")
                for ko in range(NKO):
                    nc.tensor.matmul(p_ps, lhsT=mt[:, ko, ds(jj * 128, 128)],
                                     rhs=w_proj[:, ko, :],
                                     start=(ko == 0), stop=(ko == NKO - 1))
                nc.vector.tensor_tensor(out=x[:, j, :], in0=p_ps,
                                        in1=x[:, j, :], op=ALU.add)
        mps_cm.__exit__(None, None, None)

    # ---------------- codebook head ----------------
    with tc.tile_pool(name="qm", bufs=1) as qp, \
         tc.tile_pool(name="qm2", bufs=2) as qp2:
        w_f1 = qp.tile([128, ND, C], BF16, name="wf1sb")
        w_f2 = qp.tile([128, NC_TILES, C], BF16, name="wf2sb")
        for di in range(ND):
            nc.sync.dma_start(out=w_f1[:, di, :], in_=wf1_d[ds(lD + di * 128, 128), :])
        for ci in range(NC_TILES):
            nc.sync.dma_start(out=w_f2[:, ci, :], in_=wf2_d[ds(l * C + ci * 128, 128), :])

        with tc.tile_pool(name="lnps3", bufs=2, space="PSUM") as pln3:
            mv3, r3 = _ln_stats(nc, qp2, pln3, x, "3")
            x3t = qp.tile([128, ND, NPAD], BF16, name="x3t")
            _center_transpose(nc, tc, qp2, pln3, x, mv3, r3, x3t, ident, "3")

        g = qp.tile([128, NTT * C], BF16, name="g")
        with tc.tile_pool(name="gpsp", bufs=2, space="PSUM") as gpool:
            for j in range(NTT):
                g_ps = gpool.tile([128, C], F32, name="gps", tag="gps")
                for di in range(ND):
                    nc.tensor.matmul(g_ps, lhsT=x3t[:, di, ds(j * 128, 128)],
                                     rhs=w_f1[:, di, :],
                                     start=(di == 0), stop=(di == ND - 1))
                nc.scalar.activation(out=g[:, j * C:(j + 1) * C], in_=g_ps, func=AF.Gelu)

        g2t = qp.tile([128, NC_TILES, NPAD], BF16, name="g2t")
        with tc.tile_pool(name="lnpsg", bufs=2, space="PSUM") as plng:
            mvg, rg = _ln_stats(nc, qp2, plng, g, "g", nfree=C)
            _center_transpose(nc, tc, qp2, plng, g, mvg, rg, g2t, ident, "g",
                              nfree=C, scale2=keep16)

        q2t = qp.tile([128, NC_TILES, NPAD], BF16, name="q2t")
        with tc.tile_pool(name="q2psp", bufs=2, space="PSUM") as q2pool:
            for ch in range(NPAD // 512):
                for co in range(NC_TILES):
                    q_ps = q2pool.tile([128, 512], F32, name="qps2", tag="qps2")
                    for ci in range(NC_TILES):
                        nc.tensor.matmul(q_ps, lhsT=w_f2[:, ci, ds(co * 128, 128)],
                                         rhs=g2t[:, ci, ds(ch * 512, 512)],
                                         start=(ci == 0), stop=(ci == NC_TILES - 1))
                    nc.vector.tensor_copy(out=q2t[:, co, ds(ch * 512, 512)], in_=q_ps)

        maxt = qp.tile([128, NCB, BC], F32, name="maxt")
        with tc.tile_pool(name="ipsp", bufs=3, space="PSUM") as ipool:
            for (b0, nb) in CHUNKS:
                ncols = nb * T
                for ko in range(NCB):
                    i_ps = ipool.tile([128, 6 * T], F32, name="ips", tag="ips")
                    for ci in range(NC_TILES):
                        nc.tensor.matmul(i_ps[:, :ncols],
                                         lhsT=cbt_sb[:, ci, ds(ko * 128, 128)],
                                         rhs=q2t[:, ci, ds(b0 * T, ncols)],
                                         start=(ci == 0), stop=(ci == NC_TILES - 1))
                    seg = i_ps[:, :ncols].rearrange("p (b t) -> p b t", t=T)
                    nc.vector.reduce_max(out=maxt[:, ko, b0:b0 + nb], in_=seg, axis=AX.X)

        tail_cm = tc.tile_pool(name="tailps", bufs=1, space="PSUM")
        qps = tail_cm.__enter__()
        e_sb = qp.tile([128, NCB, BC], BF16, name="esb")
        nc.scalar.activation(out=e_sb, in_=maxt, func=AF.Exp)
        s_ps = qps.tile([BC, 1], F32, name="ssps", tag="ssps")
        lu_ps = qps.tile([BC, C], F32, name="lups", tag="lups")
        for ko in range(NCB):
            nc.tensor.matmul(s_ps, lhsT=e_sb[:, ko, :], rhs=ones_c,
                             start=(ko == 0), stop=(ko == NCB - 1))
        for ko in range(NCB):
            nc.tensor.matmul(lu_ps, lhsT=e_sb[:, ko, :], rhs=cbk_sb[:, ko, :],
                             start=(ko == 0), stop=(ko == NCB - 1))
        rinv_s = qp2.tile([BC, 1], F32, name="rinvs", tag="rinvs")
        nc.vector.reciprocal(out=rinv_s, in_=s_ps)
        loc = qp2.tile([BC, C], F32, name="loc", tag="loc")
        nc.vector.tensor_scalar(out=loc, in0=lu_ps, scalar1=rinv_s,
                                scalar2=None, op0=ALU.mult)
        nc.sync.dma_start(out=local_d[l, :, :], in_=loc)
        tail_cm.__exit__(None, None, None)


# ======================= host side =======================

_BUILD_CACHE = {}


def _build(n_layers):
    if n_layers not in _BUILD_CACHE:
        nc = bacc_mod.Bacc("TRN2")
        _emit(nc, n_layers)
        nc.finalize()
        _BUILD_CACHE[n_layers] = nc
    return _BUILD_CACHE[n_layers]


def _prep_weights(inputs, n_layers):
    f = np.asarray

    def b(a):
        return np.ascontiguousarray(a.astype(np.float32)).astype(bf16)

    qkv_w = f(inputs["qkv_w"])[:n_layers]
    ln1_w = f(inputs["ln1_w"])[:n_layers]
    ln2_w = f(inputs["ln2_w"])[:n_layers]
    out_w = f(inputs["out_w"])[:n_layers]
    fc_w = f(inputs["fc_w"])[:n_layers]
    proj_w = f(inputs["proj_w"])[:n_layers]
    qm1_w = f(inputs["qm_ln1_w"])[:n_layers]
    qm2_w = f(inputs["qm_ln2_w"])[:n_layers]
    f1_w = f(inputs["qm_fc1_w"])[:n_layers]
    f2_w = f(inputs["qm_fc2_w"])[:n_layers]

    # all biases in this problem are zero and all LN scales are folded into
    # the adjacent weight matrices; assert the zero-bias assumption loudly.
    for k_ in ("ln1_b", "qkv_b", "out_b", "ln2_b", "fc_b", "proj_b",
               "qm_ln1_b", "qm_fc1_b", "qm_ln2_b", "qm_fc2_b"):
        assert not np.any(np.asarray(inputs[k_])[:n_layers]), f"nonzero {k_}"

    wqk = np.concatenate(
        [np.transpose(qkv_w[l, :2 * D, :] * ln1_w[l][None, :], (1, 0))
         for l in range(n_layers)], axis=0)                      # [L*D, 2D]
    wv = np.concatenate(
        [np.transpose(qkv_w[l, 2 * D:, :] * ln1_w[l][None, :], (1, 0))
         for l in range(n_layers)], axis=0)                      # [L*D, D]
    wout = np.concatenate(
        [np.transpose(out_w[l], (1, 0)) for l in range(n_layers)], axis=0)
    wfc = np.concatenate(
        [np.transpose(fc_w[l] * ln2_w[l][None, :], (1, 0))
         for l in range(n_layers)], axis=0)                      # [L*D, 4D]
    wproj = np.concatenate(
        [np.transpose(proj_w[l], (1, 0)) / 1.702 for l in range(n_layers)],
        axis=0)                                                  # [L*4D, D]
    wf1 = np.concatenate(
        [np.transpose(f1_w[l] * qm1_w[l][None, :], (1, 0))
         for l in range(n_layers)], axis=0)                      # [L*D, C]
    wf2 = np.concatenate(
        [np.transpose(f2_w[l] * qm2_w[l][None, :], (1, 0))
         for l in range(n_layers)], axis=0)                      # [L*C, C]

    codebook = np.asarray(inputs["codebook"], np.float32)
    lnf_w = np.asarray(inputs["lnf_w"], np.float32)
    tproj = np.asarray(inputs["text_proj"], np.float32) * lnf_w[:, None]

    # step mask in [key, query] layout: 1 where kj <= qi (kept), else 0
    mask_qk = np.triu(np.ones((T, T), np.float32))

    return {
        "wqk": b(wqk), "wv": b(wv), "wout": b(wout), "wfc": b(wfc),
        "wproj": b(wproj), "wf1": b(wf1), "wf2": b(wf2),
        "cbt": b(codebook.T), "cbk": b(codebook),
        "tproj": b(tproj),
        "ident": b(np.eye(128, dtype=np.float32)),
        "mask_qk": b(mask_qk),
        "ones_col": b(np.ones((128, 1), np.float32)),
    }


def kernel(**inputs):
    global LAST_RESULTS
    n_layers = int(os.environ.get("KERNEL_NLAYERS", L))
    n_cores = NCORES

    text = np.asarray(inputs["text"])
    tok_emb = np.asarray(inputs["tok_emb"], np.float32)
    pos_emb = np.asarray(inputs["pos_emb"], np.float32)

    eos = text.argmax(-1)                                   # [B]
    pos = np.arange(T)
    keep = ((text != 0) & (pos[None, :] != eos[:, None])).astype(np.float32)
    x0 = tok_emb[text] + pos_emb[None, :, :]                # [B,T,D] f32

    shared = _prep_weights(inputs, n_layers)

    in_maps = []
    for c in range(n_cores):
        bsl = slice(c * BC, (c + 1) * BC)
        x0c = x0[bsl].reshape(NTOK, D)
        x0c = np.concatenate([x0c, np.zeros((NPAD - NTOK, D), np.float32)], 0)
        x0c = np.ascontiguousarray(
            x0c.reshape(NTT, 128, D).transpose(1, 0, 2))    # [128, NTT, D]
        kc = keep[bsl].reshape(NTOK) / 16.0
        kc = np.concatenate([kc, np.zeros(NPAD - NTOK, np.float32)])
        kc = np.ascontiguousarray(kc.reshape(NTT, 128).transpose(1, 0))
        m = dict(shared)
        m["x0"] = x0c
        m["keep16"] = kc.astype(np.float32)
        in_maps.append(m)

    nc = _build(n_layers)
    res = run_bass_kernel_spmd(nc, in_maps, core_ids=list(range(n_cores)))
    LAST_RESULTS = res

    local_full = np.concatenate(
        [res.results[c]["local_out"] for c in range(n_cores)], axis=1)
    # pooled: gather eos rows per batch, + lnf_b correction (host-side)
    pooled = np.zeros((B, E), np.float32)
    for c in range(n_cores):
        pl = res.results[c]["pooled_out"]                   # [128, NTT, E]
        for bi in range(BC):
            b_ = c * BC + bi
            t = bi * T + int(eos[b_])
            pooled[b_] = pl[t % 128, t // 128, :]
    lnf_b = np.asarray(inputs["lnf_b"], np.float32)
    if np.any(lnf_b):
        pooled = pooled + lnf_b @ np.asarray(inputs["text_proj"], np.float32)
    return pooled, local_full


# revision 9
# speedup vs baseline: 1.0498x; 1.0498x over previous
# Trainium2 Bass kernel for nn_Clip_Codebook (CLIP-style text transformer with
# per-layer VQ-codebook cross-attention head).
#
# Sharding: data-parallel over batch B=256 across 8 NeuronCores (32 batches
# per core); codebook and all layer weights replicated.
#
# On-device layout strategy (per core):
#   - residual stream x: fp32, token-tiled SBUF [128, 20, 512]
#     (token t = j*128 + p lives at partition p, tile j; tokens 2464..2559 pad)
#   - matmul operands are produced feature-major ("xT": [D-part, token-free])
#     via PE-transpose; LayerNorm is applied token-major with fused
#     (x - mean) * rstd on the vector engine (ln scale folded into weights).
#   - attention runs per (batch, head) in a flash-style [key, query] layout:
#     softmax denominator comes from an extra N=1 matmul against ones, no
#     max-subtraction (logits are O(5) for this model family).
#   - codebook inner product is evicted from PSUM directly through a
#     segmented (per-batch) reduce_max, so the [B,T,K] tensor is never
#     materialized.
import math
import os

import numpy as np
import ml_dtypes

import concourse.bass as bass
import concourse.bacc as bacc_mod
import concourse.mybir as mybir
import concourse.tile as tile
from concourse.bass import ds
from concourse.bass_utils import run_bass_kernel_spmd

F32 = mybir.dt.float32
BF16 = mybir.dt.bfloat16
AF = mybir.ActivationFunctionType
ALU = mybir.AluOpType
AX = mybir.AxisListType
bf16 = ml_dtypes.bfloat16

# model dims
L, T, D, H, C, K, V, E, B = 12, 77, 512, 8, 256, 2048, 49408, 512, 256
DH = D // H  # 64
NEG = -1e9
EPS = 1e-5
NCORES = 8
BC = B // NCORES      # 32 batches per core
NTOK = BC * T         # 2464 real tokens per core
NTT = (NTOK + 127) // 128  # 20 token tiles
NPAD = NTT * 128      # 2560
ND = D // 128         # 4
NKO = (4 * D) // 128  # 16 fc output tiles
NCB = K // 128        # 16 codebook tiles
NC_TILES = C // 128   # 2

# batch chunks for feature-major matmuls whose PSUM is consumed with
# per-batch segmentation (6*77=462 <= 512 fp32 PSUM bank)
CHUNKS = [(0, 6), (6, 6), (12, 6), (18, 6), (24, 6), (30, 2)]

LAST_RESULTS = None  # stash of BassKernelResults for test harnesses


def _emit(nc, n_layers):
    """Emit the full per-core program into `nc`."""
    # ---------------- DRAM I/O ----------------
    x0_d = nc.dram_tensor("x0", [128, NTT, D], F32, kind="ExternalInput")
    keep16_d = nc.dram_tensor("keep16", [128, NTT], F32, kind="ExternalInput")
    wqk_d = nc.dram_tensor("wqk", [n_layers * D, 2 * D], BF16, kind="ExternalInput")
    wv_d = nc.dram_tensor("wv", [n_layers * D, D], BF16, kind="ExternalInput")
    wout_d = nc.dram_tensor("wout", [n_layers * D, D], BF16, kind="ExternalInput")
    wfc_d = nc.dram_tensor("wfc", [n_layers * D, 4 * D], BF16, kind="ExternalInput")
    wproj_d = nc.dram_tensor("wproj", [n_layers * 4 * D, D], BF16, kind="ExternalInput")
    wf1_d = nc.dram_tensor("wf1", [n_layers * D, C], BF16, kind="ExternalInput")
    wf2_d = nc.dram_tensor("wf2", [n_layers * C, C], BF16, kind="ExternalInput")
    cbt_d = nc.dram_tensor("cbt", [C, K], BF16, kind="ExternalInput")
    cbk_d = nc.dram_tensor("cbk", [K, C], BF16, kind="ExternalInput")
    tproj_d = nc.dram_tensor("tproj", [D, E], BF16, kind="ExternalInput")
    ident_d = nc.dram_tensor("ident", [128, 128], BF16, kind="ExternalInput")
    mask_d = nc.dram_tensor("mask_qk", [T, T], BF16, kind="ExternalInput")
    ones_d = nc.dram_tensor("ones_col", [128, 1], BF16, kind="ExternalInput")

    local_d = nc.dram_tensor("local_out", [n_layers, BC, C], F32, kind="ExternalOutput")
    pooled_d = nc.dram_tensor("pooled_out", [128, NTT, E], F32, kind="ExternalOutput")

    with tile.TileContext(nc) as tc:
        with tc.tile_pool(name="persist", bufs=1) as persist:
            x = persist.tile([128, NTT, D], F32, name="x")
            keep16 = persist.tile([128, NTT], F32, name="keep16sb")
            ident = persist.tile([128, 128], BF16, name="identsb")
            maskqk = persist.tile([T, T], BF16, name="masksb")
            ones_c = persist.tile([128, 1], BF16, name="onessb")
            cbt_sb = persist.tile([128, NC_TILES, K], BF16, name="cbtsb")
            cbk_sb = persist.tile([128, NCB, C], BF16, name="cbksb")
            eps_sb = persist.tile([128, 1], F32, name="epssb")
            nc.vector.memset(eps_sb, EPS)
            nc._eps_sb = eps_sb

            for j in range(NTT):
                nc.sync.dma_start(out=x[:, j, :], in_=x0_d[:, j, :])
            nc.sync.dma_start(out=keep16, in_=keep16_d[:, :])
            nc.sync.dma_start(out=ident, in_=ident_d[:, :])
            nc.sync.dma_start(out=maskqk, in_=mask_d[:, :])
            nc.sync.dma_start(out=ones_c, in_=ones_d[:, :])
            for ci in range(NC_TILES):
                nc.sync.dma_start(out=cbt_sb[:, ci, :], in_=cbt_d[ds(ci * 128, 128), :])
            for ko in range(NCB):
                nc.sync.dma_start(out=cbk_sb[:, ko, :], in_=cbk_d[ds(ko * 128, 128), :])

            # zero the pad-token region so junk can't turn into NaNs
            nc.vector.memset(x[96:128, NTT - 1, :], 0.0)

            for l in range(n_layers):
                _layer(nc, tc, l, x, keep16, ident, maskqk, ones_c, cbt_sb,
                       cbk_sb, wqk_d, wv_d, wout_d, wfc_d, wproj_d, wf1_d,
                       wf2_d, local_d)

            # ---------- final LN + text projection (all tokens) ----------
            with tc.tile_pool(name="fin", bufs=2) as fin, \
                 tc.tile_pool(name="finps", bufs=3, space="PSUM") as finps:
                tp_sb = fin.tile([128, ND, E], BF16, name="tpsb")
                for di in range(ND):
                    nc.sync.dma_start(out=tp_sb[:, di, :], in_=tproj_d[ds(di * 128, 128), :])
                mv, r_ = _ln_stats(nc, fin, finps, x, "f")
                xt = fin.tile([128, ND, NPAD], BF16, name="xft")
                _center_transpose(nc, tc, fin, finps, x, mv, r_, xt, ident, "f")
                for j in range(NTT):
                    p_ps = finps.tile([128, E], F32, name="plps", tag="plps")
                    for di in range(ND):
                        nc.tensor.matmul(p_ps, lhsT=xt[:, di, ds(j * 128, 128)],
                                         rhs=tp_sb[:, di, :],
                                         start=(di == 0), stop=(di == ND - 1))
                    po = fin.tile([128, E], F32, name="poolo", tag="poolo")
                    nc.vector.tensor_copy(out=po, in_=p_ps)
                    nc.sync.dma_start(out=pooled_d[:, j, :], in_=po)
    return nc


def _ln_stats(nc, pool, psp, src, nm, nfree=D):
    """bn_stats/bn_aggr per token tile -> (mv [128,NTT,2], r [128,NTT,1])."""
    stats = pool.tile([128, NTT, 6], F32, name=f"st{nm}", tag=f"st{nm}")
    mv = pool.tile([128, NTT, 2], F32, name=f"mv{nm}", tag=f"mv{nm}")
    r_ = pool.tile([128, NTT, 1], F32, name=f"r{nm}", tag=f"r{nm}")
    for j in range(NTT):
        if src.shape == (128, NTT, nfree):
            s_ = src[:, j, :]
        else:
            s_ = src[:, j * nfree:(j + 1) * nfree]
        nc.vector.bn_stats(out=stats[:, j, :], in_=s_)
        nc.vector.bn_aggr(out=mv[:, j, :], in_=stats[:, j, :])
    # r = 1/sqrt(var + eps)
    nc.scalar.activation(out=r_, in_=mv[:, :, 1:2], func=AF.Sqrt,
                         bias=nc._eps_sb)
    nc.vector.reciprocal(out=r_, in_=r_)
    return mv, r_


def _center_transpose(nc, tc, pool, psp, src, mv, r_, dstT, ident, nm,
                      nfree=D, scale2=None):
    """dstT[di][:, j*128:...] = transpose of ((src - mean) * r) per tile (bf16).

    scale2: optional [128, NTT] per-token extra factor folded into r.
    """
    nd = nfree // 128
    for j in range(NTT):
        xh = pool.tile([128, nfree], BF16, name=f"xh{nm}", tag=f"xh{nm}")
        if scale2 is not None:
            rr = pool.tile([128, 1], F32, name=f"rr{nm}", tag=f"rr{nm}")
            nc.vector.tensor_tensor(out=rr, in0=r_[:, j, :],
                                    in1=scale2[:, j:j + 1], op=ALU.mult)
            sc2 = rr
        else:
            sc2 = r_[:, j, :]
        src_j = src[:, j, :] if src.shape == (128, NTT, nfree) \
            else src[:, j * nfree:(j + 1) * nfree]
        nc.vector.tensor_scalar(out=xh, in0=src_j, scalar1=mv[:, j, 0:1],
                                scalar2=sc2, op0=ALU.subtract, op1=ALU.mult)
        for di in range(nd):
            t_ps = psp.tile([128, 128], BF16, name=f"tp{nm}", tag=f"tp{nm}")
            nc.tensor.transpose(t_ps, xh[:, di * 128:(di + 1) * 128], ident)
            nc.scalar.activation(out=dstT[:, di, ds(j * 128, 128)], in_=t_ps,
                                 func=AF.Copy)


def _layer(nc, tc, l, x, keep16, ident, maskqk, ones_c, cbt_sb, cbk_sb,
           wqk_d, wv_d, wout_d, wfc_d, wproj_d, wf1_d, wf2_d, local_d):
    lD = l * D

    # ---------------- attention ----------------
    with tc.tile_pool(name="attn", bufs=1) as ap, \
         tc.tile_pool(name="attn2", bufs=2) as ap2, \
         tc.tile_pool(name="attn3", bufs=3) as ap3:
        w_qk = ap.tile([128, ND, 2 * D], BF16, name="wqksb")
        w_v = ap.tile([128, ND, D], BF16, name="wvsb")
        w_out = ap.tile([128, ND, D], BF16, name="woutsb")
        for di in range(ND):
            nc.sync.dma_start(out=w_qk[:, di, :], in_=wqk_d[ds(lD + di * 128, 128), :])
            nc.sync.dma_start(out=w_v[:, di, :], in_=wv_d[ds(lD + di * 128, 128), :])
            nc.sync.dma_start(out=w_out[:, di, :], in_=wout_d[ds(lD + di * 128, 128), :])

        with tc.tile_pool(name="lnps1", bufs=4, space="PSUM") as pln:
            mv1, r1 = _ln_stats(nc, ap2, pln, x, "1")
            x1t = ap.tile([128, ND, NPAD], BF16, name="x1t")
            _center_transpose(nc, tc, ap2, pln, x, mv1, r1, x1t, ident, "1")

        oT = ap.tile([128, ND, NPAD], BF16, name="oT")
        nc.vector.memset(oT[:, :, NTOK:], 0.0)

        aps_cm = tc.tile_pool(name="atps", bufs=2, space="PSUM")
        aps1_cm = tc.tile_pool(name="atps1", bufs=1, space="PSUM")
        aps = aps_cm.__enter__()
        aps1 = aps1_cm.__enter__()
        for (b0, nb) in CHUNKS:
            ncols = nb * T
            c0 = b0 * T
            qkt = ap2.tile([128, 8, 6 * T], BF16, name="qkt", tag="qkt")
            for do in range(8):
                qk_ps = aps.tile([128, 6 * T], F32, name="qkps", tag="qkps")
                for di in range(ND):
                    nc.tensor.matmul(qk_ps[:, :ncols],
                                     lhsT=w_qk[:, di, ds(do * 128, 128)],
                                     rhs=x1t[:, di, ds(c0, ncols)],
                                     start=(di == 0), stop=(di == ND - 1))
                nc.vector.tensor_copy(out=qkt[:, do, :ncols], in_=qk_ps[:, :ncols])
            for bi in range(nb):
                b = b0 + bi
                cb0 = bi * T
                v_ps = aps1.tile([T, D], F32, name="vps", tag="vps")
                for di in range(ND):
                    nc.tensor.matmul(v_ps, lhsT=x1t[:, di, ds(b * T, T)],
                                     rhs=w_v[:, di, :],
                                     start=(di == 0), stop=(di == ND - 1))
                v_b = ap3.tile([T, H, 65], BF16, name="vb", tag="vb")
                nc.vector.tensor_copy(
                    out=v_b[:, :, 0:64],
                    in_=v_ps.rearrange("p (h d) -> p h d", d=64))
                nc.vector.memset(v_b[:, :, 64:65], 1.0)
                o_b = ap2.tile([T, D], BF16, name="ob", tag="ob")
                for h in range(H):
                    dt_ = h // 2
                    po = (h % 2) * 64
                    s_ps = aps.tile([T, T], F32, name="sps", tag="sps")
                    nc.tensor.matmul(s_ps,
                                     lhsT=qkt[po:po + 64, 4 + dt_, ds(cb0, T)],
                                     rhs=qkt[po:po + 64, dt_, ds(cb0, T)],
                                     start=True, stop=True)
                    e_t = ap3.tile([T, T], BF16, name="et", tag="et")
                    nc.scalar.activation(out=e_t, in_=s_ps, func=AF.Exp,
                                         scale=1.0 / math.sqrt(DH))
                    a_un = ap3.tile([T, T], BF16, name="aun", tag="aun")
                    nc.vector.tensor_tensor(out=a_un, in0=e_t, in1=maskqk,
                                            op=ALU.mult)
                    o_ps = aps.tile([T, 65], F32, name="ops", tag="ops")
                    nc.tensor.matmul(o_ps, lhsT=a_un, rhs=v_b[:, h, :],
                                     start=True, stop=True)
                    rinv = ap3.tile([T, 1], F32, name="rinv", tag="rinv")
                    nc.vector.reciprocal(out=rinv, in_=o_ps[:, 64:65])
                    nc.vector.tensor_scalar(out=o_b[:, h * 64:h * 64 + 64],
                                            in0=o_ps[:, 0:64], scalar1=rinv,
                                            scalar2=None, op0=ALU.mult)
                for di in range(ND):
                    t_ps = aps1.tile([128, T], BF16, name="otps", tag="otps")
                    nc.tensor.transpose(t_ps, o_b[:, di * 128:(di + 1) * 128],
                                        ident[:T, :T])
                    nc.vector.tensor_copy(out=oT[:, di, ds(b * T, T)], in_=t_ps)

        aps1_cm.__exit__(None, None, None)
        aps_cm.__exit__(None, None, None)

        # out-projection + residual
        with tc.tile_pool(name="zpsp", bufs=4, space="PSUM") as zpool:
          for j in range(NTT):
            z_ps = zpool.tile([128, D], F32, name="zps", tag="zps")
            for di in range(ND):
                nc.tensor.matmul(z_ps, lhsT=oT[:, di, ds(j * 128, 128)],
                                 rhs=w_out[:, di, :],
                                 start=(di == 0), stop=(di == ND - 1))
            nc.vector.tensor_tensor(out=x[:, j, :], in0=z_ps, in1=x[:, j, :],
                                    op=ALU.add)

    # ---------------- MLP ----------------
    with tc.tile_pool(name="mlp", bufs=1) as mp, \
         tc.tile_pool(name="mlp2", bufs=2) as mp2:
        w_fc = mp.tile([128, ND, 4 * D], BF16, name="wfcsb")
        w_proj = mp.tile([128, NKO, D], BF16, name="wprojsb")
        for di in range(ND):
            nc.sync.dma_start(out=w_fc[:, di, :], in_=wfc_d[ds(lD + di * 128, 128), :])
        for ko in range(NKO):
            nc.sync.dma_start(out=w_proj[:, ko, :],
                              in_=wproj_d[ds(l * 4 * D + ko * 128, 128), :])

        with tc.tile_pool(name="lnps2", bufs=4, space="PSUM") as pln2:
            mv2, r2 = _ln_stats(nc, mp2, pln2, x, "2")
            x2t = mp.tile([128, ND, NPAD], BF16, name="x2t")
            _center_transpose(nc, tc, mp2, pln2, x, mv2, r2, x2t, ident, "2")

        mps_cm = tc.tile_pool(name="mlpps", bufs=3, space="PSUM")
        mps = mps_cm.__enter__()
        for ch in range(NPAD // 512):
            mt = mp2.tile([128, NKO, 512], BF16, name="mt", tag="mt")
            for ko in range(NKO):
                m_ps = mps.tile([128, 512], F32, name="mps", tag="mps")
                for di in range(ND):
                    nc.tensor.matmul(m_ps, lhsT=w_fc[:, di, ds(ko * 128, 128)],
                                     rhs=x2t[:, di, ds(ch * 512, 512)],
                                     start=(di == 0), stop=(di == ND - 1))
                nc.scalar.activation(out=mt[:, ko, :], in_=m_ps, func=AF.Silu,
                                     scale=1.702)
            for jj in range(4):
                j = ch * 4 + jj
                p_ps = mps.tile([128, D], F32, name="pps", tag="pps")
                for ko in range(NKO):
                    nc.tensor.matmul(p_ps, lhsT=mt[:, ko, ds(jj * 128, 128)],
                                     rhs=w_proj[:, ko, :],
                                     start=(ko == 0), stop=(ko == NKO - 1))
                nc.vector.tensor_tensor(out=x[:, j, :], in0=p_ps,
                                        in1=x[:, j, :], op=ALU.add)
        mps_cm.__exit__(None, None, None)

    # ---------------- codebook head ----------------
    with tc.tile_pool(name="qm", bufs=1) as qp, \
         tc.tile_pool(name="qm2", bufs=2) as qp2:
        w_f1 = qp.tile([128, ND, C], BF16, name="wf1sb")
        w_f2 = qp.tile([128, NC_TILES, C], BF16, name="wf2sb")
        for di in range(ND):
            nc.sync.dma_start(out=w_f1[:, di, :], in_=wf1_d[ds(lD + di * 128, 128), :])
        for ci in range(NC_TILES):
            nc.sync.dma_start(out=w_f2[:, ci, :], in_=wf2_d[ds(l * C + ci * 128, 128), :])

        with tc.tile_pool(name="lnps3", bufs=4, space="PSUM") as pln3:
            mv3, r3 = _ln_stats(nc, qp2, pln3, x, "3")
            x3t = qp.tile([128, ND, NPAD], BF16, name="x3t")
            _center_transpose(nc, tc, qp2, pln3, x, mv3, r3, x3t, ident, "3")

        g = qp.tile([128, NTT * C], BF16, name="g")
        with tc.tile_pool(name="gpsp", bufs=4, space="PSUM") as gpool:
            for j in range(NTT):
                g_ps = gpool.tile([128, C], F32, name="gps", tag="gps")
                for di in range(ND):
                    nc.tensor.matmul(g_ps, lhsT=x3t[:, di, ds(j * 128, 128)],
                                     rhs=w_f1[:, di, :],
                                     start=(di == 0), stop=(di == ND - 1))
                nc.scalar.activation(out=g[:, j * C:(j + 1) * C], in_=g_ps, func=AF.Gelu)

        g2t = qp.tile([128, NC_TILES, NPAD], BF16, name="g2t")
        with tc.tile_pool(name="lnpsg", bufs=4, space="PSUM") as plng:
            mvg, rg = _ln_stats(nc, qp2, plng, g, "g", nfree=C)
            _center_transpose(nc, tc, qp2, plng, g, mvg, rg, g2t, ident, "g",
                              nfree=C, scale2=keep16)

        q2t = qp.tile([128, NC_TILES, NPAD], BF16, name="q2t")
        with tc.tile_pool(name="q2psp", bufs=4, space="PSUM") as q2pool:
            for ch in range(NPAD // 512):
                for co in range(NC_TILES):
                    q_ps = q2pool.tile([128, 512], F32, name="qps2", tag="qps2")
                    for ci in range(NC_TILES):
                        nc.tensor.matmul(q_ps, lhsT=w_f2[:, ci, ds(co * 128, 128)],
                                         rhs=g2t[:, ci, ds(ch * 512, 512)],
                                         start=(ci == 0), stop=(ci == NC_TILES - 1))
                    nc.scalar.activation(out=q2t[:, co, ds(ch * 512, 512)],
                                         in_=q_ps, func=AF.Copy)

        maxt = qp.tile([128, NCB, BC], F32, name="maxt")
        with tc.tile_pool(name="ipsp", bufs=4, space="PSUM") as ipool:
            for (b0, nb) in CHUNKS:
                ncols = nb * T
                for ko in range(NCB):
                    i_ps = ipool.tile([128, 6 * T], F32, name="ips", tag="ips")
                    for ci in range(NC_TILES):
                        nc.tensor.matmul(i_ps[:, :ncols],
                                         lhsT=cbt_sb[:, ci, ds(ko * 128, 128)],
                                         rhs=q2t[:, ci, ds(b0 * T, ncols)],
                                         start=(ci == 0), stop=(ci == NC_TILES - 1))
                    seg = i_ps[:, :ncols].rearrange("p (b t) -> p b t", t=T)
                    nc.vector.reduce_max(out=maxt[:, ko, b0:b0 + nb], in_=seg, axis=AX.X)

        tail_cm = tc.tile_pool(name="tailps", bufs=1, space="PSUM")
        qps = tail_cm.__enter__()
        e_sb = qp.tile([128, NCB, BC], BF16, name="esb")
        nc.scalar.activation(out=e_sb, in_=maxt, func=AF.Exp)
        s_ps = qps.tile([BC, 1], F32, name="ssps", tag="ssps")
        lu_ps = qps.tile([BC, C], F32, name="lups", tag="lups")
        for ko in range(NCB):
            nc.tensor.matmul(s_ps, lhsT=e_sb[:, ko, :], rhs=ones_c,
                             start=(ko == 0), stop=(ko == NCB - 1))
        for ko in range(NCB):
            nc.tensor.matmul(lu_ps, lhsT=e_sb[:, ko, :], rhs=cbk_sb[:, ko, :],
                             start=(ko == 0), stop=(ko == NCB - 1))
        rinv_s = qp2.tile([BC, 1], F32, name="rinvs", tag="rinvs")
        nc.vector.reciprocal(out=rinv_s, in_=s_ps)
        loc = qp2.tile([BC, C], F32, name="loc", tag="loc")
        nc.vector.tensor_scalar(out=loc, in0=lu_ps, scalar1=rinv_s,
                                scalar2=None, op0=ALU.mult)
        nc.sync.dma_start(out=local_d[l, :, :], in_=loc)
        tail_cm.__exit__(None, None, None)


# ======================= host side =======================

_BUILD_CACHE = {}


def _build(n_layers):
    if n_layers not in _BUILD_CACHE:
        nc = bacc_mod.Bacc("TRN2")
        _emit(nc, n_layers)
        nc.finalize()
        _BUILD_CACHE[n_layers] = nc
    return _BUILD_CACHE[n_layers]


def _prep_weights(inputs, n_layers):
    f = np.asarray

    def b(a):
        return np.ascontiguousarray(a.astype(np.float32)).astype(bf16)

    qkv_w = f(inputs["qkv_w"])[:n_layers]
    ln1_w = f(inputs["ln1_w"])[:n_layers]
    ln2_w = f(inputs["ln2_w"])[:n_layers]
    out_w = f(inputs["out_w"])[:n_layers]
    fc_w = f(inputs["fc_w"])[:n_layers]
    proj_w = f(inputs["proj_w"])[:n_layers]
    qm1_w = f(inputs["qm_ln1_w"])[:n_layers]
    qm2_w = f(inputs["qm_ln2_w"])[:n_layers]
    f1_w = f(inputs["qm_fc1_w"])[:n_layers]
    f2_w = f(inputs["qm_fc2_w"])[:n_layers]

    # all biases in this problem are zero and all LN scales are folded into
    # the adjacent weight matrices; assert the zero-bias assumption loudly.
    for k_ in ("ln1_b", "qkv_b", "out_b", "ln2_b", "fc_b", "proj_b",
               "qm_ln1_b", "qm_fc1_b", "qm_ln2_b", "qm_fc2_b"):
        assert not np.any(np.asarray(inputs[k_])[:n_layers]), f"nonzero {k_}"

    wqk = np.concatenate(
        [np.transpose(qkv_w[l, :2 * D, :] * ln1_w[l][None, :], (1, 0))
         for l in range(n_layers)], axis=0)                      # [L*D, 2D]
    wv = np.concatenate(
        [np.transpose(qkv_w[l, 2 * D:, :] * ln1_w[l][None, :], (1, 0))
         for l in range(n_layers)], axis=0)                      # [L*D, D]
    wout = np.concatenate(
        [np.transpose(out_w[l], (1, 0)) for l in range(n_layers)], axis=0)
    wfc = np.concatenate(
        [np.transpose(fc_w[l] * ln2_w[l][None, :], (1, 0))
         for l in range(n_layers)], axis=0)                      # [L*D, 4D]
    wproj = np.concatenate(
        [np.transpose(proj_w[l], (1, 0)) / 1.702 for l in range(n_layers)],
        axis=0)                                                  # [L*4D, D]
    wf1 = np.concatenate(
        [np.transpose(f1_w[l] * qm1_w[l][None, :], (1, 0))
         for l in range(n_layers)], axis=0)                      # [L*D, C]
    wf2 = np.concatenate(
        [np.transpose(f2_w[l] * qm2_w[l][None, :], (1, 0))
         for l in range(n_layers)], axis=0)                      # [L*C, C]

    codebook = np.asarray(inputs["codebook"], np.float32)
    lnf_w = np.asarray(inputs["lnf_w"], np.float32)
    tproj = np.asarray(inputs["text_proj"], np.float32) * lnf_w[:, None]

    # step mask in [key, query] layout: 1 where kj <= qi (kept), else 0
    mask_qk = np.triu(np.ones((T, T), np.float32))

    return {
        "wqk": b(wqk), "wv": b(wv), "wout": b(wout), "wfc": b(wfc),
        "wproj": b(wproj), "wf1": b(wf1), "wf2": b(wf2),
        "cbt": b(codebook.T), "cbk": b(codebook),
        "tproj": b(tproj),
        "ident": b(np.eye(128, dtype=np.float32)),
        "mask_qk": b(mask_qk),
        "ones_col": b(np.ones((128, 1), np.float32)),
    }


def kernel(**inputs):
    global LAST_RESULTS
    n_layers = int(os.environ.get("KERNEL_NLAYERS", L))
    n_cores = NCORES

    text = np.asarray(inputs["text"])
    tok_emb = np.asarray(inputs["tok_emb"], np.float32)
    pos_emb = np.asarray(inputs["pos_emb"], np.float32)

    eos = text.argmax(-1)                                   # [B]
    pos = np.arange(T)
    keep = ((text != 0) & (pos[None, :] != eos[:, None])).astype(np.float32)
    x0 = tok_emb[text] + pos_emb[None, :, :]                # [B,T,D] f32

    shared = _prep_weights(inputs, n_layers)

    in_maps = []
    for c in range(n_cores):
        bsl = slice(c * BC, (c + 1) * BC)
        x0c = x0[bsl].reshape(NTOK, D)
        x0c = np.concatenate([x0c, np.zeros((NPAD - NTOK, D), np.float32)], 0)
        x0c = np.ascontiguousarray(
            x0c.reshape(NTT, 128, D).transpose(1, 0, 2))    # [128, NTT, D]
        kc = keep[bsl].reshape(NTOK) / 16.0
        kc = np.concatenate([kc, np.zeros(NPAD - NTOK, np.float32)])
        kc = np.ascontiguousarray(kc.reshape(NTT, 128).transpose(1, 0))
        m = dict(shared)
        m["x0"] = x0c
        m["keep16"] = kc.astype(np.float32)
        in_maps.append(m)

    nc = _build(n_layers)
    res = run_bass_kernel_spmd(nc, in_maps, core_ids=list(range(n_cores)))
    LAST_RESULTS = res

    local_full = np.concatenate(
        [res.results[c]["local_out"] for c in range(n_cores)], axis=1)
    # pooled: gather eos rows per batch, + lnf_b correction (host-side)
    pooled = np.zeros((B, E), np.float32)
    for c in range(n_cores):
        pl = res.results[c]["pooled_out"]                   # [128, NTT, E]
        for bi in range(BC):
            b_ = c * BC + bi
            t = bi * T + int(eos[b_])
            pooled[b_] = pl[t % 128, t // 128, :]
    lnf_b = np.asarray(inputs["lnf_b"], np.float32)
    if np.any(lnf_b):
        pooled = pooled + lnf_b @ np.asarray(inputs["text_proj"], np.float32)
    return pooled, local_full
